# revision 1
# baseline (speedup 1.0000x reference)
"""CHGNetSimple GNN message passing on 8 Trainium2 NeuronCores (Bass/Tile).

Sharding: edges partitioned by src-owner core (sorted by src, fixed slot
stride per 128-atom block); triplets partitioned by k-owner core (sorted by
local k slot, fixed stride per 512-edge quad).  Sorted-key feature reads use
one-hot expand matmuls on the PE; random gathers of computed tables
(v_new[j], e_new[i]) use on-device indirect DMA; BN statistics are
AllReduced; v_new / e_new are AllGathered in bf16.  Math in bf16 with fp32
PSUM accumulation; outputs fp32.
"""
import os
import sys

for _p in ("/opt/trn_rl_repo", "/root/.axon_site/_ro/trn_rl_repo"):
    if os.path.isdir(_p) and _p not in sys.path:
        sys.path.insert(0, _p)

import numpy as np
import ml_dtypes

import concourse.bass as bass
import concourse.tile as tile
from concourse import bacc, mybir
from concourse.bass_utils import run_bass_kernel_spmd

BF16 = mybir.dt.float16  # fp16: same PE rate as bf16, 8x mantissa
FP16 = mybir.dt.float16
F32 = mybir.dt.float32
I32 = mybir.dt.int32

C = 8
D = 64
P = 128
EPS = 1e-5
bf = np.float16


def _ceil(a, b):
    return -(-a // b)


# ----------------------------------------------------------------------------
# host-side planning
# ----------------------------------------------------------------------------

def _bnp2(inputs):
    """[128, 6]: rows 0:64 = (gamma_c, beta_c) per phase, 64:128 = gate."""
    out = np.zeros((P, 6), np.float32)
    names = [("g_ac", "b_ac", "g_ag", "b_ag"), ("g_bc", "b_bc", "g_bg", "b_bg"),
             ("g_nc", "b_nc", "g_ng", "b_ng")]
    for p, (gc, bc_, gg, bg) in enumerate(names):
        out[0:D, 2 * p] = np.asarray(inputs[gc], np.float32)
        out[0:D, 2 * p + 1] = np.asarray(inputs[bc_], np.float32)
        out[D:P, 2 * p] = np.asarray(inputs[gg], np.float32)
        out[D:P, 2 * p + 1] = np.asarray(inputs[bg], np.float32)
    return out


def _plan(inputs):
    vf = np.asarray(inputs["vertex_feat"], np.float32)
    ef = np.asarray(inputs["edge_feat"], np.float32)
    af = np.asarray(inputs["angle_feat"], np.float32)
    src = np.asarray(inputs["edge_index"][0], np.int64)
    dst = np.asarray(inputs["edge_index"][1], np.int64)
    k_idx = np.asarray(inputs["k_idx"], np.int64)
    i_idx = np.asarray(inputs["i_idx"], np.int64)
    j_idx = np.asarray(inputs["j_idx"], np.int64)
    N, E, T = vf.shape[0], ef.shape[0], af.shape[0]
    assert N % C == 0
    AB = N // C
    nABlk = _ceil(AB, P)

    perm_e = np.argsort(src, kind="stable")
    src_s = src[perm_e]
    cb = np.searchsorted(src_s, np.arange(C + 1) * AB)

    stride_A = 512
    for c in range(C):
        lo, hi = cb[c], cb[c + 1]
        if hi > lo:
            blk = (src_s[lo:hi] - c * AB) // P
            stride_A = max(stride_A, int(np.bincount(blk, minlength=nABlk).max()))
    stride_A = _ceil(stride_A, 512) * 512
    EA = nABlk * stride_A

    E_c = cb[1:] - cb[:-1]
    E_pad = _ceil(int(E_c.max()), 512) * 512
    nquad = E_pad // 512

    e_core = np.empty(E, np.int64)
    e_slot = np.empty(E, np.int64)
    for c in range(C):
        lo, hi = cb[c], cb[c + 1]
        e_core[perm_e[lo:hi]] = c
        e_slot[perm_e[lo:hi]] = np.arange(hi - lo)
    t_core = e_core[k_idx]
    t_kslot = e_slot[k_idx]
    perm_t = np.lexsort((t_kslot, t_core))
    tb = np.searchsorted(t_core[perm_t], np.arange(C + 1))

    stride_B = 512
    for c in range(C):
        lo, hi = tb[c], tb[c + 1]
        if hi > lo:
            q = t_kslot[perm_t[lo:hi]] // 512
            stride_B = max(stride_B, int(np.bincount(q, minlength=nquad).max()))
    stride_B = _ceil(stride_B, P) * P
    TA = nquad * stride_B

    dims = dict(N=N, E=E, T=T, AB=AB, nABlk=nABlk, stride_A=stride_A, EA=EA,
                E_pad=E_pad, nquad=nquad, stride_B=stride_B, TA=TA)

    def t2(x):
        return np.ascontiguousarray(x.T.astype(bf))

    iota4 = np.concatenate(
        [np.tile(np.arange(P, dtype=np.float32), (P, 1)) + v * P
         for v in range(4)], axis=1)                       # [P, 4P]
    iotacol = (np.arange(P, dtype=np.float32)[:, None]
               + (np.arange(4, dtype=np.float32) * P)[None, :])  # [P, 4]
    iota1r4 = np.tile(np.tile(np.arange(P, dtype=np.float32), 4), (P, 1))

    per_core = []
    tslots = []
    for c in range(C):
        lo, hi = cb[c], cb[c + 1]
        eidx = perm_e[lo:hi]
        s_loc = src[eidx] - c * AB
        blk = s_loc // P
        cnt = np.bincount(blk, minlength=nABlk)
        off = np.arange(hi - lo) - np.concatenate(([0], np.cumsum(cnt)))[blk]
        a_slot = blk * stride_A + off
        eA = np.full(EA, -1, np.int64)
        eA[a_slot] = np.arange(hi - lo)
        vA = eA >= 0

        srcval = np.full(EA, -1.0, np.float32)
        srcval[vA] = s_loc[eA[vA]] % P

        efT_A = np.zeros((D, EA), bf)
        dstT_A = np.zeros((D, EA), bf)
        efT_A[:, vA] = t2(ef[eidx])[:, eA[vA]]
        dstT_A[:, vA] = t2(vf[dst[eidx]])[:, eA[vA]]

        vf_own = np.zeros((nABlk * P, D), bf)
        vf_own[:AB] = vf[c * AB:(c + 1) * AB].astype(bf)

        tlo, thi = tb[c], tb[c + 1]
        tidx = perm_t[tlo:thi]
        ks = t_kslot[tidx]
        q = ks // 512
        qcnt = np.bincount(q, minlength=nquad)
        toff = np.arange(thi - tlo) - np.concatenate(([0], np.cumsum(qcnt)))[q]
        t_slot = q * stride_B + toff
        tslots.append(t_slot)
        tB = np.full(TA, -1, np.int64)
        tB[t_slot] = np.arange(thi - tlo)
        vB = tB >= 0
        tv = tB[vB]

        kval = np.full(TA, -1.0, np.float32)
        kval[vB] = (ks[tv] - (np.arange(TA) // stride_B)[vB] * 512)

        angT = np.zeros((D, TA), bf)
        efkT = np.zeros((D, TA), bf)
        efiT = np.zeros((D, TA), bf)
        angT[:, vB] = t2(af[tidx])[:, tv]
        efkT[:, vB] = t2(ef[k_idx[tidx]])[:, tv]
        efiT[:, vB] = t2(ef[i_idx[tidx]])[:, tv]

        jgv = np.full(TA, N, np.int64)
        jgv[vB] = j_idx[tidx[tv]]
        igv = np.full(TA, C * E_pad, np.int64)
        igv[vB] = e_core[i_idx[tidx[tv]]] * E_pad + e_slot[i_idx[tidx[tv]]]

        ef_own = np.zeros((E_pad, D), bf)
        ef_own[:hi - lo] = ef[eidx].astype(bf)

        im = {
            "efT_A": efT_A, "dstT_A": dstT_A,
            "srcrow": srcval.reshape(1, EA).astype(np.float16),
            "segA": np.ascontiguousarray(srcval.reshape(-1, P).T),
            "vf_own": vf_own,
            "angT": angT, "efkT": efkT, "efiT": efiT,
            "kcol": np.ascontiguousarray(kval.reshape(-1, P).T),
            "krow": kval.reshape(1, TA).astype(np.float16),
            "jg": np.ascontiguousarray(jgv.reshape(-1, P).T).astype(np.int32),
            "ig": np.ascontiguousarray(igv.reshape(-1, P).T).astype(np.int32),
            "ef_own": ef_own,
            "iota4": iota4, "iotacol": iotacol, "iota1r4": iota1r4,
            "ident": np.eye(P, dtype=bf),
            "bnp2": _bnp2(inputs),
            "wc_a": np.asarray(inputs["Wc_atom"], np.float32).astype(bf),
            "wg_a": np.asarray(inputs["Wg_atom"], np.float32).astype(bf),
            "wo_a": np.asarray(inputs["Wout_atom"], np.float32).astype(bf),
            "wc_b": np.asarray(inputs["Wc_bond"], np.float32).astype(bf),
            "wg_b": np.asarray(inputs["Wg_bond"], np.float32).astype(bf),
            "wo_b": np.asarray(inputs["Wout_bond"], np.float32).astype(bf),
            "wc_n": np.asarray(inputs["Wc_ang"], np.float32).astype(bf),
            "wg_n": np.asarray(inputs["Wg_ang"], np.float32).astype(bf),
        }
        per_core.append(im)
    unshard = dict(perm_e=perm_e, perm_t=perm_t, cb=cb, tb=tb, tslots=tslots)
    return dims, per_core, unshard


# ----------------------------------------------------------------------------
# device program
# ----------------------------------------------------------------------------

def _build(dims):
    N, E, T = dims["N"], dims["E"], dims["T"]
    AB, nABlk = dims["AB"], dims["nABlk"]
    stride_A, EA = dims["stride_A"], dims["EA"]
    E_pad, nquad, SB, TA = (dims["E_pad"], dims["nquad"],
                            dims["stride_B"], dims["TA"])
    nU_A = EA // 512
    upb_A = stride_A // 512
    nsub_q = SB // P

    nc = bacc.Bacc("TRN2", target_bir_lowering=False, debug=False,
                   num_devices=C)

    def din(name, shape, dt=BF16):
        return nc.dram_tensor(name, shape, dt, kind="ExternalInput")

    efT_A = din("efT_A", [D, EA]); dstT_A = din("dstT_A", [D, EA])
    srcrow = din("srcrow", [1, EA], FP16)
    segA = din("segA", [P, EA // P], F32)
    vf_own = din("vf_own", [nABlk * P, D])
    angT = din("angT", [D, TA]); efkT = din("efkT", [D, TA])
    efiT = din("efiT", [D, TA])
    kcol = din("kcol", [P, TA // P], F32)
    krow = din("krow", [1, TA], FP16)
    jg = din("jg", [P, TA // P], I32)
    ig = din("ig", [P, TA // P], I32)
    ef_own = din("ef_own", [E_pad, D])
    iota4 = din("iota4", [P, 4 * P], F32)
    iota1r4 = din("iota1r4", [P, 4 * P], F32)
    iotacol = din("iotacol", [P, 4], F32)
    ident_in = din("ident", [P, P])
    bnp2 = din("bnp2", [P, 6], F32)
    wshapes = dict(wc_a=192, wg_a=192, wo_a=64, wc_b=256, wg_b=256, wo_b=64,
                   wc_n=256, wg_n=256)
    w_in = {n: din(n, [r, D]) for n, r in wshapes.items()}

    v_out = nc.dram_tensor("v_out", [nABlk * P, D], F32, kind="ExternalOutput")
    e_out = nc.dram_tensor("e_out", [E_pad, D], F32, kind="ExternalOutput")
    aT_out = nc.dram_tensor("aT_out", [D, TA], F32, kind="ExternalOutput")

    with tile.TileContext(nc) as tc:
        import contextlib
        stack = contextlib.ExitStack()
        cn = stack.enter_context(tc.tile_pool(name="const", bufs=1))
        dr = stack.enter_context(tc.tile_pool(name="dram", bufs=1, space="DRAM"))
        sb = stack.enter_context(tc.tile_pool(name="sb", bufs=3))
        ps = stack.enter_context(tc.tile_pool(name="ps", bufs=1, space="PSUM"))

        def load_const(name, ap, dt):
            t = cn.tile(list(ap.shape), dt, name=name)
            nc.sync.dma_start(out=t[:], in_=ap[:, :])
            return t

        iden = load_const("iden", ident_in, BF16)
        iota4_sb = load_const("iota4_sb", iota4, F32)
        iota1r4_sb = load_const("iota1r4_sb", iota1r4, F32)
        iotacol_sb = load_const("iotacol_sb", iotacol, F32)
        bnp2_sb = load_const("bnp2_sb", bnp2, F32)
        segA_sb = load_const("segA_sb", segA, F32)
        kcol_sb = load_const("kcol_sb", kcol, F32)
        jg_sb = load_const("jg_sb", jg, I32)
        ig_sb = load_const("ig_sb", ig, I32)
        W = {}
        for n, r in wshapes.items():
            parts = []
            for r0 in range(0, r, P):
                r1 = min(r0 + P, r)
                t = cn.tile([r1 - r0, D], BF16, name=f"{n}_{r0}")
                nc.sync.dma_start(out=t[:], in_=w_in[n][r0:r1, :])
                parts.append(t)
            W[n] = parts
        eps_c = cn.tile([P, 1], F32, name="eps_c")
        nc.vector.memset(eps_c[:], EPS)
        ones1 = cn.tile([1, P], FP16, name="ones1")
        nc.vector.memset(ones1[:], 1.0)
        zrow = cn.tile([16, D], BF16, name="zrow")
        nc.vector.memset(zrow[:], 0.0)

        vnew_tab = dr.tile([N + 16, D], BF16, name="vnew_tab")
        vnew_own = dr.tile([AB, D], BF16, name="vnew_own")
        enew_tab = dr.tile([C * E_pad + 16, D], BF16, name="enew_tab")
        enew_own = dr.tile([E_pad, D], BF16, name="enew_own")
        vjT_dram = dr.tile([D, TA], BF16, name="vjT_dram")
        cg_d = {ph: dr.tile([P, n], BF16, name=f"cgd_{ph}")
                for ph, n in (("A", EA), ("B", TA), ("C", TA))}
        ar_bufs = {ph: (dr.tile([P, 2], F32, name=f"ari_{ph}"),
                        dr.tile([P, 2], F32, name=f"aro_{ph}"))
                   for ph in "ABC"}

        nc.sync.dma_start(out=vnew_tab[N:N + 16, :], in_=zrow[:])
        nc.sync.dma_start(out=enew_tab[C * E_pad:C * E_pad + 16, :], in_=zrow[:])

        RG = [list(range(C))]

        # psum tags; bank budget: bc 1 + x1 2 + x2 1 + cg 2x2 = 8
        def pbc(w=512, dt=F32, p=P):
            return ps.tile([p, w], dt, space="PSUM", name="pbc", tag="bc")

        def px1(w, dt, p=D):
            return ps.tile([p, w], dt, space="PSUM", name="px1", tag="x1")

        def px2(w, dt, p=D):
            return ps.tile([p, w], dt, space="PSUM", name="px2", tag="x2")

        def pcg(w):
            return ps.tile([P, w], F32, space="PSUM", name="pcg", tag="cg",
                           bufs=2)

        def evict_cg(cgps, cgw, strips, col):
            """[128,w] psum -> fp16 sbuf; sum/sumsq strips (c rows 0:64,
            gate rows 64:128)."""
            nc.scalar.activation(cgw, cgps,
                                 mybir.ActivationFunctionType.Identity,
                                 accum_out=strips[:, col:col + 1])
            w = cgps.shape[-1]
            sq = sb.tile([P, 1024], BF16, name="sqscr", tag="sqscr")
            nc.scalar.activation(sq[:, 0:w], cgps,
                                 mybir.ActivationFunctionType.Square,
                                 accum_out=strips[:, col + 1:col + 2])

        def finalize(ph, strips, nset, count, bn_off):
            ari, aro = ar_bufs[ph]
            part = sb.tile([P, 2], F32, name=f"part{ph}", tag="stat1", bufs=1)
            nc.vector.tensor_reduce(
                out=part[:],
                in_=strips[:, 0:2 * nset].rearrange("p (b f) -> p f b", f=2),
                op=mybir.AluOpType.add, axis=mybir.AxisListType.X)
            nc.sync.dma_start(out=ari[:, :], in_=part[:])
            nc.gpsimd.collective_compute("AllReduce", mybir.AluOpType.add,
                                         replica_groups=RG,
                                         ins=[ari.opt()], outs=[aro.opt()])
            tot = sb.tile([P, 2], F32, name=f"tot{ph}", tag="stat1", bufs=1)
            nc.sync.dma_start(out=tot[:], in_=aro[:, :])
            mean = sb.tile([P, 2], F32, name=f"mean{ph}", tag="stat2", bufs=1)
            nc.scalar.activation(mean[:], tot[:],
                                 mybir.ActivationFunctionType.Identity,
                                 bias=0.0, scale=1.0 / float(count))
            var = sb.tile([P, 1], F32, name=f"var{ph}", tag="stat3", bufs=1)
            msq = sb.tile([P, 1], F32, name=f"msq{ph}", tag="stat4", bufs=1)
            nc.vector.tensor_tensor(out=msq[:], in0=mean[:, 0:1],
                                    in1=mean[:, 0:1], op=mybir.AluOpType.mult)
            nc.vector.tensor_tensor(out=var[:], in0=mean[:, 1:2], in1=msq[:],
                                    op=mybir.AluOpType.subtract)
            std = sb.tile([P, 1], F32, name=f"std{ph}", tag="stat5", bufs=1)
            nc.scalar.activation(std[:], var[:],
                                 mybir.ActivationFunctionType.Sqrt,
                                 bias=eps_c[:, 0:1], scale=1.0)
            rstd = sb.tile([P, 1], F32, name=f"rstd{ph}", tag="stat6", bufs=1)
            nc.vector.reciprocal(rstd[:], std[:])
            sc = cn.tile([P, 1], F32, name=f"sc{ph}")
            bi = cn.tile([P, 1], F32, name=f"bi{ph}")
            nc.vector.tensor_tensor(out=sc[:], in0=rstd[:],
                                    in1=bnp2_sb[:, bn_off:bn_off + 1],
                                    op=mybir.AluOpType.mult)
            tmp = sb.tile([P, 1], F32, name=f"tmp{ph}", tag="stat7", bufs=1)
            nc.vector.tensor_tensor(out=tmp[:], in0=mean[:, 0:1], in1=sc[:],
                                    op=mybir.AluOpType.mult)
            nc.vector.tensor_tensor(out=bi[:],
                                    in0=bnp2_sb[:, bn_off + 1:bn_off + 2],
                                    in1=tmp[:], op=mybir.AluOpType.subtract)
            sc2 = cn.tile([D, 1], F32, name=f"sc2{ph}")
            bi2 = cn.tile([D, 1], F32, name=f"bi2{ph}")
            nc.sync.dma_start(out=sc2[:], in_=sc[D:P, :])
            nc.sync.dma_start(out=bi2[:], in_=bi[D:P, :])
            return sc, bi, sc2, bi2

        def bn_gate2(cd, o, sc, bi, sc2_, bi2_, w, tagp):
            """load c/g halves (base-0 each) from cd[:, o:o+w]; -> mT [64,w]."""
            cw = sb.tile([D, w], BF16, name="cw", tag=tagp + "cw")
            gw = sb.tile([D, w], BF16, name="gw", tag=tagp + "gw")
            nc.sync.dma_start(out=cw[:], in_=cd[0:D, o:o + w])
            nc.sync.dma_start(out=gw[:], in_=cd[D:P, o:o + w])
            bnc = sb.tile([D, w], BF16, name="bnc", tag=tagp + "bnc")
            sgc = sb.tile([D, w], BF16, name="sgc", tag=tagp + "sgc")
            sgg = sb.tile([D, w], BF16, name="sgg", tag=tagp + "sgg")
            nc.scalar.activation(bnc[:], cw[:],
                                 mybir.ActivationFunctionType.Identity,
                                 bias=bi[0:D, :], scale=sc[0:D, :])
            nc.scalar.activation(sgc[:], cw[:],
                                 mybir.ActivationFunctionType.Sigmoid,
                                 bias=bi[0:D, :], scale=sc[0:D, :])
            nc.scalar.activation(sgg[:], gw[:],
                                 mybir.ActivationFunctionType.Sigmoid,
                                 bias=bi2_[:, 0:1], scale=sc2_[:, 0:1])
            m1 = sb.tile([D, w], BF16, name="m1", tag=tagp + "m1")
            mT = sb.tile([D, w], BF16, name="mT", tag=tagp + "mT")
            nc.vector.tensor_tensor(out=m1[:], in0=bnc[:],
                                    in1=sgc[:], op=mybir.AluOpType.mult)
            nc.vector.tensor_tensor(out=mT[:], in0=m1[:], in1=sgg[:],
                                    op=mybir.AluOpType.mult)
            return mT

        # =====================================================
        # PHASE A
        # =====================================================
        strips_A = cn.tile([P, 2 * nU_A], F32, name="strips_A")
        cdA = cg_d["A"]
        for u in range(nU_A):
            o = 512 * u
            g = o // stride_A
            srow = sb.tile([1, 512], FP16, name="srow", tag="srow")
            nc.sync.dma_start(out=srow[:], in_=srcrow[:, o:o + 512])
            bc = pbc()
            nc.tensor.matmul(out=bc[:], lhsT=ones1[:], rhs=srow[:],
                             start=True, stop=True)
            pm = sb.tile([P, 512], BF16, name="pmA", tag="pmA")
            nc.vector.tensor_tensor(
                out=pm[:], in0=bc[:],
                in1=iotacol_sb[:, 0:1].to_broadcast([P, 512]),
                op=mybir.AluOpType.is_equal)
            vblk = sb.tile([P, D], BF16, name="vblkA", tag="vblkA")
            nc.sync.dma_start(out=vblk[:], in_=vf_own[g * P:(g + 1) * P, :])
            xps = px1(512, F32)
            nc.tensor.matmul(out=xps[:], lhsT=vblk[:], rhs=pm[:],
                             start=True, stop=True)
            t1 = sb.tile([P, 512], BF16, name="t1A", tag="t1A")
            nc.vector.tensor_copy(out=t1[0:D, :], in_=xps[:])
            nc.sync.dma_start(out=t1[D:P, :], in_=efT_A[:, o:o + 512])
            t2_ = sb.tile([D, 512], BF16, name="t2A", tag="t2A")
            nc.sync.dma_start(out=t2_[:], in_=dstT_A[:, o:o + 512])
            cgps = pcg(512)
            for wn, r0 in (("wc_a", 0), ("wg_a", D)):
                nc.tensor.matmul(out=cgps[r0:r0 + D, :], lhsT=W[wn][0][:],
                                 rhs=t1[:], start=True, stop=False)
                nc.tensor.matmul(out=cgps[r0:r0 + D, :], lhsT=W[wn][1][:],
                                 rhs=t2_[:], start=False, stop=True)
            cgw = sb.tile([P, 512], BF16, name="cgwA", tag="cgwA")
            evict_cg(cgps[:], cgw[:], strips_A, 2 * u)
            nc.sync.dma_start(out=cdA[:, o:o + 512], in_=cgw[:])

        scA, biA, scA2, biA2 = finalize("A", strips_A, nU_A, E, 0)

        for g in range(nABlk):
            sps = px2(P, F32)
            for ui in range(upb_A):
                u = g * upb_A + ui
                o = 512 * u
                mT = bn_gate2(cdA, o, scA, biA, scA2, biA2, 512, "A")
                mnp = px1(4 * D, BF16, p=P)
                for s4 in range(4):
                    nc.tensor.transpose(out=mnp[:, s4 * D:(s4 + 1) * D],
                                        in_=mT[:, s4 * P:(s4 + 1) * P],
                                        identity=iden[0:D, 0:D])
                mn = sb.tile([P, 4 * D], BF16, name="mnA", tag="mnA")
                nc.vector.tensor_copy(out=mn[:], in_=mnp[:])
                s = 4 * u
                pseg4 = sb.tile([P, 512], BF16, name="pseg4A", tag="pseg4A")
                nc.vector.tensor_tensor(
                    out=pseg4[:].rearrange("p (s j) -> p s j", s=4),
                    in0=segA_sb[:, s:s + 4].to_broadcast([P, 4, P]),
                    in1=iota1r4_sb[:].rearrange("p (s j) -> p s j", s=4),
                    op=mybir.AluOpType.is_equal)
                for s4 in range(4):
                    nc.tensor.matmul(out=sps[:],
                                     lhsT=mn[:, s4 * D:(s4 + 1) * D],
                                     rhs=pseg4[:, s4 * P:(s4 + 1) * P],
                                     start=(ui == 0 and s4 == 0),
                                     stop=(ui == upb_A - 1 and s4 == 3))
            sT = sb.tile([D, P], BF16, name="sTA", tag="sTA")
            nc.vector.tensor_copy(out=sT[:], in_=sps[:])
            vps = px1(D, F32, p=P)
            nc.tensor.matmul(out=vps[:], lhsT=sT[:], rhs=W["wo_a"][0][:],
                             start=True, stop=True)
            vblk = sb.tile([P, D], BF16, name="vresA", tag="vresA")
            nc.sync.dma_start(out=vblk[:], in_=vf_own[g * P:(g + 1) * P, :])
            vnf = sb.tile([P, D], F32, name="vnfA", tag="vnfA")
            nc.vector.tensor_tensor(out=vnf[:], in0=vps[:], in1=vblk[:],
                                    op=mybir.AluOpType.add)
            nc.sync.dma_start(out=v_out[g * P:(g + 1) * P, :], in_=vnf[:])
            vnb = sb.tile([P, D], BF16, name="vnbA", tag="vnbA")
            nc.vector.tensor_copy(out=vnb[:], in_=vnf[:])
            r0, r1 = g * P, min((g + 1) * P, AB)
            if r1 > r0:
                nc.sync.dma_start(out=vnew_own[r0:r1, :], in_=vnb[0:r1 - r0, :])

        nc.gpsimd.collective_compute("AllGather", mybir.AluOpType.bypass,
                                     replica_groups=RG,
                                     ins=[vnew_own.opt()],
                                     outs=[vnew_tab[0:N, :].opt()])

        # =====================================================
        # PHASE B
        # =====================================================
        strips_B = cn.tile([P, 2 * nquad], F32, name="strips_B")
        cdB = cg_d["B"]
        for q in range(nquad):
            o = q * SB
            jrows = sb.tile([P, nsub_q * D], BF16, name="jrB", tag="jrB")
            jtp = px1(SB, BF16)
            for s in range(nsub_q):
                gi = q * nsub_q + s
                nc.gpsimd.indirect_dma_start(
                    out=jrows[:, s * D:(s + 1) * D], out_offset=None,
                    in_=vnew_tab[:],
                    in_offset=bass.IndirectOffsetOnAxis(
                        ap=jg_sb[:, gi:gi + 1], axis=0))
                nc.tensor.transpose(out=jtp[:, s * P:(s + 1) * P],
                                    in_=jrows[:, s * D:(s + 1) * D],
                                    identity=iden[:, :])
            t1 = sb.tile([P, SB], BF16, name="t1B", tag="t1B")
            nc.vector.tensor_copy(out=t1[0:D, :], in_=jtp[:])
            nc.sync.dma_start(out=vjT_dram[:, o:o + SB], in_=t1[0:D, :])
            nc.sync.dma_start(out=t1[D:P, :], in_=efkT[:, o:o + SB])
            t2_ = sb.tile([P, SB], BF16, name="t2B", tag="t2B")
            nc.sync.dma_start(out=t2_[0:D, :], in_=efiT[:, o:o + SB])
            nc.sync.dma_start(out=t2_[D:P, :], in_=angT[:, o:o + SB])
            cgps = pcg(SB)
            for wn, r0 in (("wc_b", 0), ("wg_b", D)):
                for x0 in range(0, SB, 512):
                    x1 = min(x0 + 512, SB)
                    nc.tensor.matmul(out=cgps[r0:r0 + D, x0:x1],
                                     lhsT=W[wn][0][:], rhs=t1[:, x0:x1],
                                     start=True, stop=False)
                    nc.tensor.matmul(out=cgps[r0:r0 + D, x0:x1],
                                     lhsT=W[wn][1][:], rhs=t2_[:, x0:x1],
                                     start=False, stop=True)
            cgw = sb.tile([P, SB], BF16, name="cgwB", tag="cgwB")
            evict_cg(cgps[:], cgw[:], strips_B, 2 * q)
            nc.sync.dma_start(out=cdB[:, o:o + SB], in_=cgw[:])

        scB, biB, scB2, biB2 = finalize("B", strips_B, nquad, T, 2)

        for q in range(nquad):
            o = q * SB
            mT = bn_gate2(cdB, o, scB, biB, scB2, biB2, SB, "B")
            mnp = px1(nsub_q * D, BF16, p=P)
            for s in range(nsub_q):
                nc.tensor.transpose(out=mnp[:, s * D:(s + 1) * D],
                                    in_=mT[:, s * P:(s + 1) * P],
                                    identity=iden[0:D, 0:D])
            mn = sb.tile([P, nsub_q * D], BF16, name="mnB", tag="mnB")
            nc.vector.tensor_copy(out=mn[:], in_=mnp[:])
            seg4 = px2(4 * P, F32)
            for s in range(nsub_q):
                gi = q * nsub_q + s
                p4 = sb.tile([P, 4 * P], BF16, name="p4B", tag="p4B")
                nc.vector.tensor_tensor(
                    out=p4[:],
                    in0=kcol_sb[:, gi:gi + 1].to_broadcast([P, 4 * P]),
                    in1=iota4_sb[:], op=mybir.AluOpType.is_equal)
                nc.tensor.matmul(out=seg4[:], lhsT=mn[:, s * D:(s + 1) * D],
                                 rhs=p4[:], start=(s == 0),
                                 stop=(s == nsub_q - 1))
            sT4 = sb.tile([D, 4 * P], BF16, name="sT4B", tag="sT4B")
            nc.vector.tensor_copy(out=sT4[:], in_=seg4[:])
            eps4 = px1(4 * D, F32, p=P)
            for si in range(4):
                nc.tensor.matmul(out=eps4[:, si * D:(si + 1) * D],
                                 lhsT=sT4[:, si * P:(si + 1) * P],
                                 rhs=W["wo_b"][0][:], start=True, stop=True)
            ebk4 = sb.tile([P, 4 * D], BF16, name="ebk4B", tag="ebk4B")
            nc.sync.dma_start(
                out=ebk4[:].rearrange("p (s d) -> p s d", s=4),
                in_=ef_own[4 * q * P:4 * (q + 1) * P, :]
                    .rearrange("(s p) d -> p s d", s=4))
            enf4 = sb.tile([P, 4 * D], F32, name="enf4B", tag="enf4B")
            nc.vector.tensor_tensor(out=enf4[:], in0=eps4[:], in1=ebk4[:],
                                    op=mybir.AluOpType.add)
            nc.sync.dma_start(
                out=e_out[4 * q * P:4 * (q + 1) * P, :]
                    .rearrange("(s p) d -> p s d", s=4),
                in_=enf4[:].rearrange("p (s d) -> p s d", s=4))
            enb4 = sb.tile([P, 4 * D], BF16, name="enb4B", tag="enb4B")
            nc.vector.tensor_copy(out=enb4[:], in_=enf4[:])
            nc.sync.dma_start(
                out=enew_own[4 * q * P:4 * (q + 1) * P, :]
                    .rearrange("(s p) d -> p s d", s=4),
                in_=enb4[:].rearrange("p (s d) -> p s d", s=4))

        nc.gpsimd.collective_compute("AllGather", mybir.AluOpType.bypass,
                                     replica_groups=RG,
                                     ins=[enew_own.opt()],
                                     outs=[enew_tab[0:C * E_pad, :].opt()])

        # =====================================================
        # PHASE C
        # =====================================================
        strips_C = cn.tile([P, 2 * nquad], F32, name="strips_C")
        cdC = cg_d["C"]
        for q in range(nquad):
            o = q * SB
            ekb4 = sb.tile([P, 4 * D], BF16, name="ekb4C", tag="ekb4C")
            nc.sync.dma_start(
                out=ekb4[:].rearrange("p (s d) -> p s d", s=4),
                in_=enew_own[4 * q * P:4 * (q + 1) * P, :]
                    .rearrange("(s p) d -> p s d", s=4))
            ekp = px1(SB, F32)
            for x0 in range(0, SB, 512):
                x1 = min(x0 + 512, SB)
                krw = sb.tile([1, 512], FP16, name="krwC", tag="krwC")
                nc.sync.dma_start(out=krw[:, 0:x1 - x0],
                                  in_=krow[:, o + x0:o + x1])
                bc = pbc()
                nc.tensor.matmul(out=bc[:, 0:x1 - x0], lhsT=ones1[:],
                                 rhs=krw[:, 0:x1 - x0], start=True, stop=True)
                for si in range(4):
                    pm = sb.tile([P, 512], BF16, name="pmC", tag="pmC")
                    nc.vector.tensor_tensor(
                        out=pm[:, 0:x1 - x0], in0=bc[:, 0:x1 - x0],
                        in1=iotacol_sb[:, si:si + 1].to_broadcast(
                            [P, x1 - x0]),
                        op=mybir.AluOpType.is_equal)
                    nc.tensor.matmul(out=ekp[:, x0:x1],
                                     lhsT=ekb4[:, si * D:(si + 1) * D],
                                     rhs=pm[:, 0:x1 - x0], start=(si == 0),
                                     stop=(si == 3))
            t1 = sb.tile([P, SB], BF16, name="t1C", tag="t1C")
            nc.sync.dma_start(out=t1[0:D, :], in_=vjT_dram[:, o:o + SB])
            nc.vector.tensor_copy(out=t1[D:P, :], in_=ekp[:])
            irows = sb.tile([P, nsub_q * D], BF16, name="irC", tag="irC")
            itp = px2(SB, BF16)
            for s in range(nsub_q):
                gi = q * nsub_q + s
                nc.gpsimd.indirect_dma_start(
                    out=irows[:, s * D:(s + 1) * D], out_offset=None,
                    in_=enew_tab[:],
                    in_offset=bass.IndirectOffsetOnAxis(
                        ap=ig_sb[:, gi:gi + 1], axis=0))
                nc.tensor.transpose(out=itp[:, s * P:(s + 1) * P],
                                    in_=irows[:, s * D:(s + 1) * D],
                                    identity=iden[:, :])
            t2_ = sb.tile([P, SB], BF16, name="t2C", tag="t2C")
            nc.vector.tensor_copy(out=t2_[0:D, :], in_=itp[:])
            nc.sync.dma_start(out=t2_[D:P, :], in_=angT[:, o:o + SB])
            cgps = pcg(SB)
            for wn, r0 in (("wc_n", 0), ("wg_n", D)):
                for x0 in range(0, SB, 512):
                    x1 = min(x0 + 512, SB)
                    nc.tensor.matmul(out=cgps[r0:r0 + D, x0:x1],
                                     lhsT=W[wn][0][:], rhs=t1[:, x0:x1],
                                     start=True, stop=False)
                    nc.tensor.matmul(out=cgps[r0:r0 + D, x0:x1],
                                     lhsT=W[wn][1][:], rhs=t2_[:, x0:x1],
                                     start=False, stop=True)
            cgw = sb.tile([P, SB], BF16, name="cgwC", tag="cgwC")
            evict_cg(cgps[:], cgw[:], strips_C, 2 * q)
            nc.sync.dma_start(out=cdC[:, o:o + SB], in_=cgw[:])

        scC, biC, scC2, biC2 = finalize("C", strips_C, nquad, T, 4)

        for q in range(nquad):
            o = q * SB
            mT = bn_gate2(cdC, o, scC, biC, scC2, biC2, SB, "C")
            ang2 = sb.tile([D, SB], BF16, name="ang2C", tag="ang2C")
            nc.sync.dma_start(out=ang2[:], in_=angT[:, o:o + SB])
            aT = sb.tile([D, SB], F32, name="aTC", tag="aTC")
            nc.vector.tensor_tensor(out=aT[:], in0=mT[:], in1=ang2[:],
                                    op=mybir.AluOpType.add)
            nc.sync.dma_start(out=aT_out[:, o:o + SB], in_=aT[:])

        stack.close()
    nc.finalize()
    return nc


# ----------------------------------------------------------------------------
# public entry point
# ----------------------------------------------------------------------------

_CACHE = {}


def kernel(**inputs):
    dims, per_core, unshard = _plan(inputs)
    key = tuple(sorted(dims.items()))
    if key not in _CACHE:
        _CACHE[key] = _build(dims)
    nc = _CACHE[key]

    res = run_bass_kernel_spmd(nc, per_core, core_ids=list(range(C)))

    N, E, T, AB = dims["N"], dims["E"], dims["T"], dims["AB"]
    perm_e, perm_t, cb, tb, tslots = (unshard["perm_e"], unshard["perm_t"],
                                      unshard["cb"], unshard["tb"],
                                      unshard["tslots"])
    v_new = np.empty((N, D), np.float32)
    e_new = np.empty((E, D), np.float32)
    a_new = np.empty((T, D), np.float32)
    for c in range(C):
        r = res.results[c]
        v_new[c * AB:(c + 1) * AB] = r["v_out"][:AB]
        lo, hi = cb[c], cb[c + 1]
        e_new[perm_e[lo:hi]] = r["e_out"][:hi - lo]
        tlo, thi = tb[c], tb[c + 1]
        a_new[perm_t[tlo:thi]] = r["aT_out"][:, tslots[c]].T
    return np.concatenate([v_new, e_new, a_new], axis=0).astype(np.float32)

